# revision 32
# baseline (speedup 1.0000x reference)
"""Trainium2 Bass kernel for nn_DenseSparsePreEmbedding.

Math refactoring (verified bit-exact vs the jax reference on CPU):
    fixed_emb @ W_fixed  == (fixed_table @ W_fixed)[fixed_features]
    sparse_emb @ W_sparse== (concat(tabs) @ W_sparse)[cv]  with cv the
                            combined per-token sparse code (last write wins,
                            sentinel 256 -> zero row for untouched tokens)
so the whole module collapses to a dual embedding gather + add:
    out[n] = tabA[ffn] + tabB[cvn]
with tabA = fixed_table @ W_fixed + b   [2048, 128] f32
     tabB = concat(tab0..3) @ W_sparse (+ zero row)  [257, 128] f32

Device kernel (SPMD over 8 cores, 125000 tokens each), final config:
  ILV=1 BF16=1 REP=64 TT=512 — per 512-token tile ONE interleaved
  gpsimd.dma_gather of 1024 bf16 256B rows from the combined HBM table
  (A' ++ 64 replicas of Btilde), DVE block-pair add (bf16+bf16->f32),
  HWDGE store. 589 us NEFF exec (traced), rel err 2.8e-3 (bf16 tables).

Optimization history (traced Core-0 NEFF times):
  1645 us  staged baseline (2 f32 gathers/tile, 4 SWDGE queues)
   692 us  +REP=64: replicate the 257-row B table 64x in HBM. 61% of
           tokens hit the sentinel row; those reads serialized on one
           512B HBM region (B-gather engines ran at 3.5 vs 16.6 GB/s).
   647 us  +BF16 tables: halves gather read bytes (rel err 2.8e-3,
           gate is 2e-2).
   589 us  +ILV: one interleaved gather/tile instead of two. Pool-engine
           descriptor generation is THE bottleneck: measured ucode cost
           ~230ns/instruction + ~2.0ns/gathered row (the v2 cost model's
           994ns + 0.34ns/desc is wrong on HW). 250k rows/core -> ~560us
           pool floor; wall runs at ~96% pool busy.

Hard limits learned on HW this session:
  - one dma_gather wedges the DEVICE (NRT_EXEC_UNIT_UNRECOVERABLE) above
    ~1024 idxs (1280 fails, 1024 ok; f32/bf16, REP=1/4/64 all same), so
    ILV TT=512 (2*512=1024 idxs) is the max tile. KSCR (dynamic dma
    scratch) did not lift it.
  - indirect_dma_start (HW-DGE vector-indirect, qPoolDynamic) is BROKEN
    under bass native lowering: offset stream is misread (stateful
    garbage past the first partition row) and odd shapes wedge the
    device. Verified correct in CoreSim only. Do not use.
  - PE one-hot B-lookup (bc-matmul + is_equal + 2 accumulating matmuls)
    is numerically exact but too slow: 712ns/matmul instr (x3/tile) and
    DVE-from-PSUM at ~75G els/s -> both engines alone exceed the SWDGE
    pool floor. Benched in pe_bench.py.
  - gpsimd ap_gather/indirect_copy use per-16-partition-GROUP shared
    indices (not per-partition) -> unusable for per-token lookups in
    [token, dim] layout.
"""

import numpy as np

N = 1_000_000
NCORES = 8
PER = N // NCORES          # 125000 tokens per core
V = 2048
D = 128
NSPARSE = 257              # 4*64 sparse rows + zero sentinel row
import os as _os

TT = int(_os.environ.get("KTT", "512"))     # tokens per tile (ring limit: <=1920 idx/op)
NQUEUES = int(_os.environ.get("KNQ", "4"))  # SWDGE queues to spread gathers over
_scr = _os.environ.get("KSCR", "")
SCRATCH = int(_scr) if _scr else None  # dynamic_dma_scratch_size (None = default 16KB)
BUFS = int(_os.environ.get("KBUFS", "8"))   # work tile-pool buffers
ILV = int(_os.environ.get("KILV", "1"))     # interleaved single-gather mode
SP = bool(int(_os.environ.get("KSP", "1"))) # dma_gather single_packet flag
SSPLIT = int(_os.environ.get("KSSPLIT", "0"))  # alternate stores sync/scalar HWDGE
PERM = int(_os.environ.get("KPERM", "0"))   # tile-transposed token order -> 2KB stores
QASYM = int(_os.environ.get("KQASYM", "0"))  # A-gathers on queues 0-2, B on queue 3
B2 = int(_os.environ.get("KB2", "0"))       # one 2x-wide B-gather per two tiles
CHUNK = int(_os.environ.get("KCHUNK", "8")) # split idx preloads into N chunks
REP = int(_os.environ.get("KREP", "64"))    # B-table HBM replicas (hot-row spread)
BF16 = int(_os.environ.get("KBF16", "1"))   # bf16 tables (halves gather HBM bytes)
OB16 = int(_os.environ.get("KOB16", "0"))   # bf16 device output (host widens to f32)
MODE = _os.environ.get("KMODE", "swdge")    # swdge = dma_gather; ind = HW-DGE indirect (BROKEN on HW: bass native lowering misreads the offset stream and can wedge the device)
ITT = int(_os.environ.get("KITT", "1024"))  # tokens per tile in indirect mode
# (measured: PERM=1 cut store packets 4x but slowed gathers 17% — coarser
#  store packets block gather interleave at the SDMA engines. Keep 0.)
NSPR = NSPARSE * REP                        # replicated B rows in HBM
NTAB = V + NSPR                             # combined table rows (A' ++ Btilde reps)
PAD = 125056               # per-core tokens padded (tile grid, mult of 2048)
COLS = PAD // 16           # 7816 wrapped-index columns

_cache = {}


def _build_nc(per_core=PER, tt=TT, nqueues=NQUEUES, scratch=SCRATCH, bufs=3):
    import concourse.bacc as bacc
    import concourse.mybir as mybir
    import concourse.tile as tile

    nfull = per_core // tt
    tailv = per_core - nfull * tt
    tailp = ((tailv + 127) // 128) * 128
    pad = nfull * tt + tailp
    cols = pad // 16

    kw = {} if scratch is None else {"dynamic_dma_scratch_size": scratch}
    if nqueues > 1:
        kw["num_swdge_queues"] = nqueues
    nc = bacc.Bacc(
        "TRN2",
        target_bir_lowering=False,
        debug=False,
        enable_asserts=False,
        **kw,
    )
    tdt = mybir.dt.bfloat16 if BF16 else mybir.dt.float32
    idxa_t = nc.dram_tensor("idxa", [128, cols], mybir.dt.int16, kind="ExternalInput")
    idxb_t = nc.dram_tensor("idxb", [128, cols], mybir.dt.int16, kind="ExternalInput")
    taba_t = nc.dram_tensor("taba", [V, D], tdt, kind="ExternalInput")
    tabb_t = nc.dram_tensor("tabb", [NSPR, D], tdt, kind="ExternalInput")
    out_t = nc.dram_tensor("out", [per_core, D], mybir.dt.float32, kind="ExternalOutput")

    idxa = idxa_t.ap()
    idxb = idxb_t.ap()
    taba = taba_t.ap()
    tabb = tabb_t.ap()
    out = out_t.ap()

    with tile.TileContext(nc) as tc:
        with (
            tc.tile_pool(name="idxp", bufs=1) as ip,
            tc.tile_pool(name="work", bufs=bufs) as wp,
        ):
            ia = ip.tile([128, cols], mybir.dt.int16, tag="ia")
            ib = ip.tile([128, cols], mybir.dt.int16, tag="ib")
            if CHUNK > 1:
                # chunked preload: first gathers only wait on their own chunk
                step = (cols + CHUNK - 1) // CHUNK
                for c0_ in range(0, cols, step):
                    c1_ = min(c0_ + step, cols)
                    nc.sync.dma_start(out=ia[:, c0_:c1_], in_=idxa[:, c0_:c1_])
                    nc.sync.dma_start(out=ib[:, c0_:c1_], in_=idxb[:, c0_:c1_])
            else:
                nc.sync.dma_start(out=ia[:], in_=idxa)
                nc.sync.dma_start(out=ib[:], in_=idxb)

            ntiles = nfull + (1 if tailp else 0)
            db2 = None
            for t in range(ntiles):
                ni = tt if t < nfull else tailp      # gathered (padded) tokens
                valid = tt if t < nfull else tailv   # rows actually stored
                nblk = (ni + 127) // 128
                c0 = (t * tt) // 16
                da = wp.tile([128, nblk, 128], tdt, tag="da")
                fo = wp.tile([128, nblk, 128], mybir.dt.float32, tag="fo")
                if QASYM and nqueues == 4:
                    # the 1MB A-table is read-latency-bound: give it 3 rings;
                    # the hot 128KB B-table (61% sentinel-row hits) gets 1.
                    qa = t % 3
                    qb = 3
                elif nqueues > 1:
                    qa = (2 * t) % nqueues
                    qb = (2 * t + 1) % nqueues
                else:
                    qa = qb = 0
                nc.gpsimd.dma_gather(
                    da[:], taba, ia[:, c0 : c0 + ni // 16], ni, ni, D,
                    queue_num=qa, single_packet=SP,
                )
                if B2 and t + 1 < nfull and t % 2 == 0:
                    # one 2x-wide B gather feeds this tile and the next
                    db2 = wp.tile([128, 2 * nblk, 128], tdt, tag="db")
                    nc.gpsimd.dma_gather(
                        db2[:], tabb, ib[:, c0 : c0 + 2 * ni // 16], 2 * ni,
                        2 * ni, D, queue_num=qb, single_packet=SP,
                    )
                    dbv = db2[:, :nblk, :]
                elif B2 and t % 2 == 1 and t < nfull:
                    dbv = db2[:, nblk : 2 * nblk, :]
                else:
                    db = wp.tile([128, nblk, 128], tdt, tag="db")
                    nc.gpsimd.dma_gather(
                        db[:], tabb, ib[:, c0 : c0 + ni // 16], ni, ni, D,
                        queue_num=qb, single_packet=SP,
                    )
                    dbv = db[:]
                nc.vector.tensor_add(out=fo[:], in0=da[:], in1=dbv)
                r0 = t * tt
                fb = valid // 128
                rem = valid - fb * 128
                st = nc.scalar if (SSPLIT and t % 2) else nc.sync
                if PERM and t < nfull:
                    # host permuted this tile's stream so stream slot b*128+p
                    # carries token p*fb+b: partition p's store is fb
                    # consecutive rows = one contiguous fb*512B chunk.
                    ov = out[r0 : r0 + tt, :].rearrange("(p b) e -> p b e", b=fb)
                    st.dma_start(out=ov, in_=fo[:, :fb, :])
                    continue
                if fb:
                    ov = out[r0 : r0 + fb * 128, :].rearrange(
                        "(b p) e -> p b e", p=128
                    )
                    st.dma_start(out=ov, in_=fo[:, :fb, :])
                if rem:
                    ov2 = out[r0 + fb * 128 : r0 + valid, :].rearrange(
                        "(b p) e -> p b e", p=rem
                    )
                    st.dma_start(out=ov2, in_=fo[:rem, fb : fb + 1, :])
    nc.compile()
    return nc


def _build_nc_ind(per_core=PER, tt=ITT, bufs=BUFS):
    """Hardware-DGE indirect-gather mode.

    gpsimd.indirect_dma_start (qPoolDynamic) gathers table rows with the
    offsets read by the DMA hardware itself -> no per-row SWDGE descriptor
    generation on the Q7 cores (which capped the dma_gather version).

    Token (r0 + p*fb + j) maps to SBUF [p, j] so each partition stores fb
    consecutive 512B output rows as one contiguous chunk (PERM-for-free).
    Offsets are int32 row indices, host-arranged as [128, ncols] with one
    column group of fb per full tile (+1 tail column).
    """
    import concourse.bacc as bacc
    import concourse.bass as bassmod
    import concourse.mybir as mybir
    import concourse.tile as tile

    assert tt % 128 == 0
    fb = tt // 128
    nfull = per_core // tt
    tailv = per_core - nfull * tt            # 125000 - 244*512... valid tail tokens
    assert tailv < 128 * fb
    tailc = (tailv + 127) // 128             # tail columns (tokens p*tailc+j... natural)
    cols = nfull * fb + tailc

    nc = bacc.Bacc(
        "TRN2",
        target_bir_lowering=False,
        debug=False,
        enable_asserts=False,
    )
    tdt = mybir.dt.bfloat16 if BF16 else mybir.dt.float32
    oa_t = nc.dram_tensor("oa", [128, cols], mybir.dt.int32, kind="ExternalInput")
    ob_t = nc.dram_tensor("ob", [128, cols], mybir.dt.int32, kind="ExternalInput")
    taba_t = nc.dram_tensor("taba", [V, D], tdt, kind="ExternalInput")
    tabb_t = nc.dram_tensor("tabb", [NSPR, D], tdt, kind="ExternalInput")
    out_t = nc.dram_tensor("out", [per_core, D], mybir.dt.float32, kind="ExternalOutput")

    oa = oa_t.ap()
    ob = ob_t.ap()
    taba = taba_t.ap()
    tabb = tabb_t.ap()
    out = out_t.ap()

    with tile.TileContext(nc) as tc:
        with (
            tc.tile_pool(name="idxp", bufs=1) as ip,
            tc.tile_pool(name="work", bufs=bufs) as wp,
        ):
            osa = ip.tile([128, cols], mybir.dt.int32, tag="oa")
            osb = ip.tile([128, cols], mybir.dt.int32, tag="ob")
            if CHUNK > 1:
                step = (cols + CHUNK - 1) // CHUNK
                for c0_ in range(0, cols, step):
                    c1_ = min(c0_ + step, cols)
                    nc.sync.dma_start(out=osa[:, c0_:c1_], in_=oa[:, c0_:c1_])
                    nc.sync.dma_start(out=osb[:, c0_:c1_], in_=ob[:, c0_:c1_])
            else:
                nc.sync.dma_start(out=osa[:], in_=oa)
                nc.sync.dma_start(out=osb[:], in_=ob)

            ntiles = nfull + (1 if tailc else 0)
            for t in range(ntiles):
                nb = fb if t < nfull else tailc
                c0 = t * fb
                da = wp.tile([128, nb, 128], tdt, tag="da")
                db = wp.tile([128, nb, 128], tdt, tag="db")
                fo = wp.tile([128, nb, 128], mybir.dt.float32, tag="fo")
                nc.gpsimd.indirect_dma_start(
                    out=da[:],
                    out_offset=None,
                    in_=taba,
                    in_offset=bassmod.IndirectOffsetOnAxis(
                        ap=osa[:, c0 : c0 + nb], axis=0
                    ),
                )
                nc.gpsimd.indirect_dma_start(
                    out=db[:],
                    out_offset=None,
                    in_=tabb,
                    in_offset=bassmod.IndirectOffsetOnAxis(
                        ap=osb[:, c0 : c0 + nb], axis=0
                    ),
                )
                nc.vector.tensor_add(out=fo[:], in0=da[:], in1=db[:])
                r0 = t * tt
                if t < nfull:
                    ov = out[r0 : r0 + tt, :].rearrange("(p b) e -> p b e", b=fb)
                    nc.sync.dma_start(out=ov, in_=fo[:])
                else:
                    # tail: token r0 + p*tailc + j, only `tailv` rows valid
                    full_p = tailv // tailc      # partitions with all tailc rows
                    ov = out[r0 : r0 + full_p * tailc, :].rearrange(
                        "(p b) e -> p b e", b=tailc
                    )
                    nc.sync.dma_start(out=ov, in_=fo[:full_p, :, :])
                    rem = tailv - full_p * tailc
                    if rem:
                        ov2 = out[
                            r0 + full_p * tailc : r0 + tailv, :
                        ].rearrange("(p b) e -> p b e", b=rem)
                        nc.sync.dma_start(
                            out=ov2, in_=fo[full_p : full_p + 1, :rem, :]
                        )
    nc.compile()
    return nc


def _build_nc_ilv(per_core=PER, ttok=896, nqueues=NQUEUES, scratch=SCRATCH, bufs=BUFS):
    """Interleaved mode: one dma_gather per tile from the combined table.
    Index stream per 128-token chunk: [ff(128), cv+2048(128)], so gathered
    blocks alternate A/B on the same partitions; DVE adds block-pairs."""
    import concourse.bacc as bacc
    import concourse.mybir as mybir
    import concourse.tile as tile

    assert ttok % 128 == 0
    nfull = per_core // ttok
    tailv = per_core - nfull * ttok          # valid tail tokens
    tailp = ((tailv + 127) // 128) * 128     # padded tail tokens
    pad = nfull * ttok + tailp
    nidx = 2 * pad                           # interleaved index count
    cols = nidx // 16

    kw = {} if scratch is None else {"dynamic_dma_scratch_size": scratch}
    if nqueues > 1:
        kw["num_swdge_queues"] = nqueues
    nc = bacc.Bacc(
        "TRN2", target_bir_lowering=False, debug=False, enable_asserts=False, **kw
    )
    tdt = mybir.dt.bfloat16 if BF16 else mybir.dt.float32
    odt = mybir.dt.bfloat16 if OB16 else mybir.dt.float32
    idx_t = nc.dram_tensor("idxab", [128, cols], mybir.dt.int16, kind="ExternalInput")
    tab_t = nc.dram_tensor("tabab", [NTAB, D], tdt, kind="ExternalInput")
    out_t = nc.dram_tensor("out", [per_core, D], odt, kind="ExternalOutput")
    idx = idx_t.ap()
    tab = tab_t.ap()
    out = out_t.ap()

    with tile.TileContext(nc) as tc:
        with (
            tc.tile_pool(name="idxp", bufs=1) as ip,
            tc.tile_pool(name="work", bufs=bufs) as wp,
        ):
            isb = ip.tile([128, cols], mybir.dt.int16, tag="i")
            if CHUNK > 1:
                # chunked preload: first gathers only wait on their own chunk
                step = (cols + CHUNK - 1) // CHUNK
                for c0_ in range(0, cols, step):
                    c1_ = min(c0_ + step, cols)
                    nc.sync.dma_start(out=isb[:, c0_:c1_], in_=idx[:, c0_:c1_])
            else:
                nc.sync.dma_start(out=isb[:], in_=idx)
            ntiles = nfull + (1 if tailp else 0)
            op = 0
            for t in range(ntiles):
                tok = ttok if t < nfull else tailp
                ni = 2 * tok
                npair = tok // 128
                c0 = (2 * ttok // 16) * t
                q = op % nqueues if nqueues > 1 else 0
                op += 1
                g4 = wp.tile([128, npair, 2, 128], tdt, tag="g")
                cmp = wp.tile([128, npair, 128], odt, tag="c")
                gv = g4[:].rearrange("p a b e -> p (a b) e")
                nc.gpsimd.dma_gather(
                    gv, tab, isb[:, c0 : c0 + ni // 16], ni, ni, D, queue_num=q
                )
                nc.vector.tensor_add(
                    out=cmp[:], in0=g4[:, :, 0, :], in1=g4[:, :, 1, :]
                )
                r0 = t * ttok
                valid = tok if t < nfull else tailv
                fb = valid // 128
                rem = valid - fb * 128
                if SSPLIT == 2:
                    st = nc.scalar
                elif SSPLIT == 1:
                    st = nc.scalar if t % 2 else nc.sync
                else:
                    st = nc.sync
                if PERM and t < nfull:
                    # host permuted the stream: pair-slot b*128+p carries
                    # token p*npair+b -> each partition stores npair
                    # consecutive 512B rows as one contiguous chunk
                    ov = out[r0 : r0 + tok, :].rearrange(
                        "(p b) e -> p b e", b=npair
                    )
                    st.dma_start(out=ov, in_=cmp[:])
                    continue
                if fb:
                    ov = out[r0 : r0 + fb * 128, :].rearrange(
                        "(b p) e -> p b e", p=128
                    )
                    st.dma_start(out=ov, in_=cmp[:, :fb, :])
                if rem:
                    ov2 = out[r0 + fb * 128 : r0 + valid, :].rearrange(
                        "(b p) e -> p b e", p=rem
                    )
                    st.dma_start(out=ov2, in_=cmp[:rem, fb : fb + 1, :])
    nc.compile()
    return nc


def _make_offsets(flat_i32, tt):
    """[PER] int32 row indices -> [128, cols] indirect-offset layout:
    token r0 + p*fb + j of full tile t lands at [p, t*fb + j]."""
    fb = tt // 128
    nfull = PER // tt
    tailv = PER - nfull * tt
    tailc = (tailv + 127) // 128
    cols = nfull * fb + tailc
    o = np.zeros((128, cols), np.int32)
    head = flat_i32[: nfull * tt].reshape(nfull, 128, fb)
    o[:, : nfull * fb] = head.transpose(1, 0, 2).reshape(128, nfull * fb)
    tail = np.zeros(128 * tailc, np.int32)
    tail[:tailv] = flat_i32[nfull * tt :]
    o[:, nfull * fb :] = tail.reshape(128, tailc)
    return np.ascontiguousarray(o)


def _get_nc():
    if "nc" not in _cache:
        if MODE == "ind":
            _cache["nc"] = _build_nc_ind(per_core=PER, tt=ITT, bufs=BUFS)
        elif ILV:
            _cache["nc"] = _build_nc_ilv(
                per_core=PER, ttok=TT, nqueues=NQUEUES, scratch=SCRATCH, bufs=BUFS
            )
        else:
            _cache["nc"] = _build_nc(
                per_core=PER, tt=TT, nqueues=NQUEUES, scratch=SCRATCH, bufs=BUFS
            )
    return _cache["nc"]


def _permute_tiles(arr, tt, nfull):
    """Transpose token order within each full tile so that gather stream slot
    b*128+p carries token p*(tt//128)+b — makes per-partition store chunks
    contiguous. Tail (blk=1 effective) is left in natural order."""
    blk = tt // 128
    if blk <= 1 or nfull == 0:
        return arr
    out = arr.copy()
    head = arr[: nfull * tt].reshape(nfull, 128, blk)
    out[: nfull * tt] = head.transpose(0, 2, 1).reshape(nfull * tt)
    return out


def _wrap_idx(arr_i16):
    """[PAD] int16 -> [128, COLS] dma_gather wrapped layout: index i lives at
    [i % 16, i // 16]; the 16-row block is replicated to fill 128 partitions."""
    w16 = arr_i16.reshape(-1, 16).T  # [16, COLS]
    return np.ascontiguousarray(np.tile(w16, (8, 1)))  # [128, COLS]


def kernel(
    fixed_features,
    idx0, val0, idx1, val1, idx2, val2, idx3, val3,
    fixed_table, tab0, tab1, tab2, tab3, W_fixed, W_sparse, b,
):
    from concourse.bass_utils import run_bass_kernel_spmd

    ff = np.asarray(fixed_features)
    # combined sparse code per token; 256 = untouched sentinel (zero row).
    cv = np.full(N, 256, dtype=np.int32)
    for k, (ii, vv) in enumerate(
        ((idx0, val0), (idx1, val1), (idx2, val2), (idx3, val3))
    ):
        cv[np.asarray(ii)] = k * 64 + np.asarray(vv).astype(np.int32)

    ft = np.asarray(fixed_table, dtype=np.float32)
    wf = np.asarray(W_fixed, dtype=np.float32)
    ws = np.asarray(W_sparse, dtype=np.float32)
    bb = np.asarray(b, dtype=np.float32)
    taba = (ft @ wf + bb).astype(np.float32)
    tabs = np.concatenate(
        [np.asarray(t, dtype=np.float32) for t in (tab0, tab1, tab2, tab3)], axis=0
    )
    tabb = np.concatenate([tabs @ ws, np.zeros((1, D), np.float32)], axis=0)
    tabb = np.ascontiguousarray(tabb.astype(np.float32))
    if REP > 1:
        # replicate the hot 257-row table so the 61%-hit sentinel row spreads
        # over REP distinct HBM regions (bank-conflict fix; values identical)
        tabb = np.ascontiguousarray(np.tile(tabb, (REP, 1)))
    if BF16:
        import ml_dtypes

        taba = np.ascontiguousarray(taba.astype(ml_dtypes.bfloat16))
        tabb = np.ascontiguousarray(tabb.astype(ml_dtypes.bfloat16))

    nfull = PER // TT
    tailp = ((PER - nfull * TT + 127) // 128) * 128
    padt = nfull * TT + tailp
    if ILV:
        tabab = np.ascontiguousarray(np.concatenate([taba, tabb], axis=0))
    in_maps = []
    for c in range(NCORES):
        sl = slice(c * PER, (c + 1) * PER)
        if MODE == "ind":
            ffc = ff[sl].astype(np.int32)
            cvc = cv[sl].astype(np.int32)
            if REP > 1:
                cvc = cvc + 257 * (np.arange(PER, dtype=np.int32) % REP)
            in_maps.append(
                {
                    "oa": _make_offsets(ffc, ITT),
                    "ob": _make_offsets(cvc, ITT),
                    "taba": taba,
                    "tabb": tabb,
                }
            )
            continue
        if ILV:
            fa = np.zeros(padt, dtype=np.int16)
            fa[:PER] = ff[sl].astype(np.int16)
            fbv = np.full(padt, 256 + 2048, dtype=np.int16)
            fbv[:PER] = cv[sl].astype(np.int16) + 2048
            if REP > 1:
                fbv += (257 * (np.arange(padt, dtype=np.int64) % REP)).astype(
                    np.int16
                )
            if PERM:
                fa = _permute_tiles(fa, TT, nfull)
                fbv = _permute_tiles(fbv, TT, nfull)
            seq = np.stack(
                [fa.reshape(-1, 128), fbv.reshape(-1, 128)], axis=1
            ).reshape(-1)
            in_maps.append({"idxab": _wrap_idx(seq), "tabab": tabab})
            continue
        fa = np.zeros(padt, dtype=np.int16)
        fa[:PER] = ff[sl].astype(np.int16)
        fbv = np.full(padt, 256, dtype=np.int16)
        fbv[:PER] = cv[sl].astype(np.int16)
        if REP > 1:
            fbv += (257 * (np.arange(padt, dtype=np.int64) % REP)).astype(np.int16)
        if PERM:
            fa = _permute_tiles(fa, TT, nfull)
            fbv = _permute_tiles(fbv, TT, nfull)
        in_maps.append(
            {
                "idxa": _wrap_idx(fa),
                "idxb": _wrap_idx(fbv),
                "taba": taba,
                "tabb": tabb,
            }
        )

    nc = _get_nc()
    res = run_bass_kernel_spmd(nc, in_maps, core_ids=list(range(NCORES)))
    _cache["last_results"] = res
    out = np.concatenate([res.results[c]["out"] for c in range(NCORES)], axis=0)
    if out.dtype != np.float32:
        out = out.astype(np.float32)
    return out

